# revision 59
# baseline (speedup 1.0000x reference)
"""Causal self-attention (B=2, T=2048, C=1024, H=16, D=64) on 8 trn2 cores.

Sharding: core = b*4 + hg  (data-parallel over batch b, tensor-parallel over
4 head-groups of 4 heads).  Each core computes q/k/v projections for its
256 head-dims, flash-style causal attention for its 4 heads, and a partial
output projection (its 256-column slice of Wp).  Partials are summed on the
host (the all-reduce), bias bp added there too.

Device layout notes:
  - All matmuls run in bfloat16 (measured 1.0 PE cycles/row on HW vs
    ~1.5 for float32r); partial outputs stored bf16 and summed f32 on
    the host.  Worst-case rel err ~4e-3 vs the 2e-2 gate.
  - Inputs are host-packed partition-major so every load is ONE
    contiguous dma_start (4-8KB descriptor rows).  Each dma_start costs
    ~600ns of serial HWDGE issue, so issue COUNT - not bytes - set the
    old startup latency.  All bulk loads ride the Sync ring in strict
    priority order (tri, x-tile0, wq, ...); ring FIFO gives earlier
    transfers full DMA bandwidth.
  - The PE HAM clock gate starts at 1.2GHz and doubles only after
    ~3.4us of sustained activity; dummy warm-up matmuls on tri burn the
    DMA head so the real stream starts at full clock, and filler
    matmuls in the thin epilogue keep it there.
  - qT/kT stored as [d', t] with 2 heads packed per 128 partitions.
  - v stored naturally [t, d'] with a ones-column appended per head
    (65 cols/head) so the PV matmul's output row 64 is the softmax
    denominator l[q] for free; the ones are GPSIMD memsets.
  - S^T tiles [k=128, q=512] land in PSUM, exp on ACT (scale=1/8 fused,
    no max-subtraction: |S|<~8 so exp is safe).  Diagonal chunks are
    column-restricted to the causal range; only the 128-wide triangle
    needs a mask multiply (DVE - GPSIMD tensor ops sem-stall the
    pipeline, ~600ns per event).
  - One flat software pipeline runs across all q-tiles (no per-j drain):
    S matmuls for group g+1 are emitted before exp/PV of group g, heads
    of a pair alternate, and projections for tile t+1 plus the previous
    tile's out-projections are woven in as PE filler.
  - Normalization: 1/l on DVE row 0, partition-broadcast on GPSIMD,
    DVE mul into yT - the PE stays out of it entirely.
"""
import numpy as np
from contextlib import ExitStack

B, T, C, H, D = 2, 2048, 1024, 16, 64
HLOC = 4            # heads per core
CLOC = HLOC * D     # 256 head-dims per core
VW = HLOC * 65      # v width with ones-columns: 260
N_CORES = 8
TQ = 512            # q tile width
KC = 128            # k chunk
NCC = C // 128      # 8 contraction chunks

MM_DT = "bfloat16"  # fp32r measures 1.5 cyc/row on HW; bf16 is 1.0

_CACHE = {}


def build_nc(with_qk_bias=True, with_v_bias=False):
    import concourse.tile as tile
    from concourse import bacc, mybir

    f32 = mybir.dt.float32
    fmm = getattr(mybir.dt, MM_DT)
    Exp = mybir.ActivationFunctionType.Exp

    nc = bacc.Bacc("TRN2", target_bir_lowering=False, debug=False,
                   num_devices=N_CORES)
    # Host-packed partition-major layouts: each input is a [128, *] tensor
    # whose rows match the SBUF destination exactly, so every load is ONE
    # dma_start with contiguous 1-4KB descriptor runs.  (Each dma_start
    # costs ~600ns of serial HWDGE issue time regardless of size, so issue
    # count — not bytes — governed the old kernel's startup latency.)
    xT0 = nc.dram_tensor("xT0", [128, NCC * TQ], fmm, kind="ExternalInput").ap()
    xTr = nc.dram_tensor("xTr", [128, NCC * (T - TQ)], fmm,
                         kind="ExternalInput").ap()
    wqP = nc.dram_tensor("wqP", [128, NCC * CLOC], fmm, kind="ExternalInput").ap()
    wkP = nc.dram_tensor("wkP", [128, NCC * CLOC], fmm, kind="ExternalInput").ap()
    wvP = nc.dram_tensor("wvP", [128, NCC * VW], fmm, kind="ExternalInput").ap()
    wpP = nc.dram_tensor("wpP", [128, 2 * C], fmm, kind="ExternalInput").ap()
    bq = nc.dram_tensor("bq", [1, CLOC], fmm, kind="ExternalInput").ap()
    bk = nc.dram_tensor("bk", [1, CLOC], fmm, kind="ExternalInput").ap()
    bv = nc.dram_tensor("bv", [1, VW], fmm, kind="ExternalInput").ap()
    ones = nc.dram_tensor("ones", [1, TQ], fmm, kind="ExternalInput").ap()
    tri = nc.dram_tensor("tri", [KC, KC], fmm, kind="ExternalInput").ap()
    po = nc.dram_tensor("po", [T, C], fmm, kind="ExternalOutput").ap()

    with tile.TileContext(nc) as tc, ExitStack() as ctx:
        persist = ctx.enter_context(tc.tile_pool(name="persist", bufs=1))
        pt_pool = ctx.enter_context(tc.tile_pool(name="pt", bufs=4))
        stage = ctx.enter_context(tc.tile_pool(name="stage", bufs=2))
        norm_pool = ctx.enter_context(tc.tile_pool(name="norm", bufs=4))
        ps_small = ctx.enter_context(
            tc.tile_pool(name="ps_small", bufs=2, space="PSUM"))
        ps_st = ctx.enter_context(
            tc.tile_pool(name="ps_st", bufs=2, space="PSUM"))
        ps_yt = ctx.enter_context(
            tc.tile_pool(name="ps_yt", bufs=2, space="PSUM"))

        # ---- persistent SBUF tensors + loads ----
        # xT_all columns: t-block-major [tb][c][t%TQ] — the whole tensor is
        # a contiguous copy of the (likewise packed) DRAM xT0/xTr, giving
        # full-row DMA descriptors; matmul rhs slices stay contiguous.
        xT_all = persist.tile([128, NCC * T], fmm, tag="xT", name="xT_all")
        wq_all = persist.tile([128, NCC * CLOC], fmm, tag="wq", name="wq_all")
        wk_all = persist.tile([128, NCC * CLOC], fmm, tag="wk", name="wk_all")
        wv_all = persist.tile([128, NCC * VW], fmm, tag="wv", name="wv_all")
        wp_all = persist.tile([128, 2 * C], fmm, tag="wp", name="wp_all")
        bq_sb = persist.tile([1, CLOC], fmm, tag="bq")
        bk_sb = persist.tile([1, CLOC], fmm, tag="bk")
        bv_sb = persist.tile([1, VW], fmm, tag="bv")
        ones_sb = persist.tile([1, TQ], fmm, tag="ones")
        tri_sb = persist.tile([KC, KC], fmm, tag="tri")
        qT_sb = [persist.tile([128, T], fmm, tag=f"qT{m}", name=f"qT{m}") for m in range(2)]
        kT_sb = [persist.tile([128, T], fmm, tag=f"kT{m}", name=f"kT{m}") for m in range(2)]
        v_sb = [persist.tile([128, VW], fmm, tag=f"v{t}", name=f"v{t}") for t in range(T // 128)]
        yT_sb = [persist.tile([128, T], fmm, tag=f"yT{m}", name=f"yT{m}") for m in range(2)]

        def xsl(c, pos, width):
            """xT_all slice for chunk c, x columns [pos, pos+width) —
            must stay within one TQ tile."""
            tb, off = divmod(pos, TQ)
            base = tb * NCC * TQ + c * TQ + off
            return xT_all[:, base:base + width]

        # All bulk loads go on ONE ring (Sync) in strict priority order —
        # ring FIFO means earlier transfers get full DMA bandwidth instead
        # of competing with later ones.  Only the tiny constants ride the
        # Scalar ring concurrently.
        HW = NCC * TQ // 2
        QH = NCC * CLOC // 2
        nc.sync.dma_start(tri_sb[:], tri[:])   # tiny; feeds PE warm-up MMs
        nc.sync.dma_start(xT_all[:, 0:HW], xT0[:, 0:HW])
        nc.sync.dma_start(wq_all[:, 0:QH], wqP[:, 0:QH])
        nc.sync.dma_start(wq_all[:, QH:2 * QH], wqP[:, QH:2 * QH])
        nc.sync.dma_start(xT_all[:, HW:2 * HW], xT0[:, HW:2 * HW])
        nc.sync.dma_start(wk_all[:], wkP[:, :])
        nc.sync.dma_start(wv_all[:], wvP[:, :])
        W = NCC * TQ
        for tb in range(1, T // TQ):
            nc.sync.dma_start(xT_all[:, tb * W:(tb + 1) * W],
                              xTr[:, (tb - 1) * W:tb * W])
        nc.sync.dma_start(wp_all[:], wpP[:, :])
        if with_v_bias:
            nc.scalar.dma_start(bv_sb[:], bv[:])
        if with_qk_bias or with_v_bias:
            nc.scalar.dma_start(ones_sb[:], ones[:])
        if with_qk_bias:
            nc.scalar.dma_start(bq_sb[:], bq[:])
            nc.scalar.dma_start(bk_sb[:], bk[:])

        # ---- interleaved emission: projections / attention / out-proj ----
        # The PE executes its queue in order, so emission order controls PE
        # density.  Attention for q-tile j only needs projections up to
        # t=j, so projections for t=j+1 and the out-projection for j-1 are
        # woven between attention groups of j to fill PE idle slots (keeps
        # the HAM clock-gate warm).
        def proj_qk(w_all, b_sb, dst, m, t):
            tsl = slice(t * TQ, (t + 1) * TQ)
            ps = ps_small.tile([128, TQ], f32, tag="ps_small")
            for c in range(NCC):
                nc.tensor.matmul(ps[:],
                                 w_all[:, c * CLOC + m * 128:
                                       c * CLOC + (m + 1) * 128],
                                 xsl(c, t * TQ, TQ),
                                 start=(c == 0),
                                 stop=(c == NCC - 1 and not with_qk_bias))
            if with_qk_bias:
                nc.tensor.matmul(ps[:], b_sb[0:1, m * 128:(m + 1) * 128],
                                 ones_sb[0:1, :],
                                 start=False, stop=True)
            nc.vector.tensor_copy(dst[m][:, tsl], ps[:])

        def proj_v(tt):
            ps = ps_small.tile([128, VW], f32, tag="ps_small")
            for c in range(NCC):
                nc.tensor.matmul(ps[:],
                                 xsl(c, tt * 128, 128),
                                 wv_all[:, c * VW:(c + 1) * VW],
                                 start=(c == 0),
                                 stop=(c == NCC - 1 and not with_v_bias))
            if with_v_bias:
                # supplies the ones-columns + v bias via the PE
                nc.tensor.matmul(ps[:], ones_sb[0:1, 0:128], bv_sb[:],
                                 start=False, stop=True)
            nc.vector.tensor_copy(v_sb[tt][:], ps[:])
            if not with_v_bias:
                # ones-columns written by the (otherwise idle) GPSIMD
                for h in range(HLOC):
                    nc.gpsimd.memset(v_sb[tt][:, h * 65 + 64:h * 65 + 65], 1.0)

        def proj_pieces(t):
            out = []
            for w_all, b_sb, dst in ((wq_all, bq_sb, qT_sb), (wk_all, bk_sb, kT_sb)):
                for m in range(2):
                    out.append(lambda w=w_all, b=b_sb, d=dst, mm=m:
                               proj_qk(w, b, d, mm, t))
            for tt in range(t * 4, t * 4 + 4):
                out.append(lambda x=tt: proj_v(x))
            return out

        def outproj_piece(tt, epilogue=False):
            ttsl = slice(tt * 128, (tt + 1) * 128)
            so = stage.tile([128, C], fmm, tag="so")
            for do in range(2):
                dsl = slice(do * TQ, (do + 1) * TQ)
                # epilogue: allocate from the attention-idle ps_st pool so
                # PE->DVE copy WAR chains don't throttle the final pieces
                pool = ps_st if epilogue else ps_small
                ops = pool.tile([128, TQ], f32,
                                tag="st" if epilogue else "ps_small")
                for m2 in range(2):
                    nc.tensor.matmul(ops[:], yT_sb[m2][:, ttsl],
                                     wp_all[:, m2 * C + do * TQ:
                                            m2 * C + (do + 1) * TQ],
                                     start=(m2 == 0), stop=(m2 == 1))
                nc.vector.tensor_copy(so[:, dsl], ops[:])
            # mid-run stores stay on Sync: a Scalar-issued store would
            # serialize behind queued EXP work and stall the PV pipeline.
            eng = nc.scalar if (epilogue and tt % 2) else nc.sync
            eng.dma_start(po[ttsl, :], so[:])

        def outproj_pieces_epi(j):
            return [lambda t=tt: outproj_piece(t, epilogue=True)
                    for tt in range(4 * j, 4 * j + 4)]

        def outproj_pieces(j):
            return [lambda t=tt: outproj_piece(t)
                    for tt in range(4 * j, 4 * j + 4)]

        def s_group(j, h, kcs):
            """Emit S matmuls for a k-chunk pair; return (st_tile, info)."""
            m, pr = h // 2, (h % 2) * 64
            st = ps_st.tile([128, 1024], f32, tag="st")
            info = []
            for i, kc in enumerate(kcs):
                coff = max(0, kc * KC - j * TQ)   # causal column offset
                nc.tensor.matmul(
                    st[:, i * TQ + coff:(i + 1) * TQ],
                    kT_sb[m][pr:pr + 64, kc * KC:(kc + 1) * KC],
                    qT_sb[m][pr:pr + 64, j * TQ + coff:(j + 1) * TQ],
                    start=True, stop=True)
                info.append((i, kc, coff))
            return st, info

        def pv_group(j, h, st, info, yt, nk):
            """exp + triangle mask + PV matmuls for a prepared S group."""
            pt = pt_pool.tile([128, 1024], fmm, tag="pt")
            runs = []
            for i, kc, coff in info:
                lo, hi = i * TQ + coff, (i + 1) * TQ
                if runs and runs[-1][1] == lo:
                    runs[-1][1] = hi
                else:
                    runs.append([lo, hi])
            for lo, hi in runs:
                nc.scalar.activation(pt[:, lo:hi], st[:, lo:hi], Exp, scale=0.125)
            for i, kc, coff in info:
                if kc >= 4 * j:   # diagonal chunk: mask the 128-wide triangle
                    lo = i * TQ + coff
                    nc.vector.tensor_mul(pt[:, lo:lo + KC], pt[:, lo:lo + KC],
                                         tri_sb[:])
            for i, kc, coff in info:
                lo = i * TQ + coff
                nc.tensor.matmul(
                    yt[0:65, coff:TQ] if coff else yt[:],
                    v_sb[kc][:, h * 65:(h + 1) * 65],
                    pt[:, lo:(i + 1) * TQ],
                    start=(kc == 0), stop=(kc == nk - 1))

        def normalize(j, h, yt):
            """yT[h slice, j] = yt[0:64] * broadcast(1/l).

            1/l on DVE (row 0 only), partition-broadcast on the idle
            GPSIMD — keeps the PE out of the normalization entirely.
            """
            m, pr = h // 2, (h % 2) * 64
            l_sb = norm_pool.tile([1, TQ], f32, tag="l")
            nc.vector.tensor_copy(l_sb[:], yt[64:65, :])
            r_sb = norm_pool.tile([1, TQ], f32, tag="r")
            nc.vector.reciprocal_approx_fast(r_sb[:], l_sb[:])
            bc_sb = stage.tile([64, TQ], f32, tag="bc")
            nc.gpsimd.partition_broadcast(bc_sb[:], r_sb[:])
            nc.vector.tensor_mul(yT_sb[m][pr:pr + 64, j * TQ:(j + 1) * TQ],
                                 yt[0:64, :], bc_sb[:])

        def normalize_drain(items):
            """Final normalizes: every epilogue m1 matmul waits on these, so
            latency matters.  Interleave the heads' chains so the GPSIMD
            broadcasts hide behind the other head's DVE ops (the DVE queue
            is strict FIFO — a serial emission stalls it on each
            broadcast)."""
            ls, rs, bcs = [], [], []
            for (j, h, yt) in items:
                l_sb = norm_pool.tile([1, TQ], f32, tag="l")
                nc.vector.tensor_copy(l_sb[:], yt[64:65, :])
                ls.append(l_sb)
            for l_sb in ls:
                r_sb = norm_pool.tile([1, TQ], f32, tag="r")
                nc.vector.reciprocal_approx_fast(r_sb[:], l_sb[:])
                rs.append(r_sb)
            for r_sb in rs:
                bc_sb = stage.tile([64, TQ], f32, tag="bc")
                nc.gpsimd.partition_broadcast(bc_sb[:], r_sb[:])
                bcs.append(bc_sb)
            for (j, h, yt), bc_sb in zip(items, bcs):
                m, pr = h // 2, (h % 2) * 64
                nc.vector.tensor_mul(
                    yT_sb[m][pr:pr + 64, j * TQ:(j + 1) * TQ],
                    yt[0:64, :], bc_sb[:])

        # ---- PE warm-up ----
        # The HAM clock gate needs ~3.4us of sustained PE activity before it
        # un-throttles (1.2 -> 2.4 GHz).  The PE sits idle waiting for input
        # DMA until ~13us; burn that window with dummy matmuls on tri_sb
        # (first tensor to land) so the real stream starts warm.
        warm_ps = ps_small.tile([128, KC], f32, tag="ps_small", name="warm")
        for _ in range(36):
            nc.tensor.matmul(warm_ps[:], tri_sb[:], tri_sb[:],
                             start=True, stop=True)

        for piece in proj_pieces(0):    # prologue
            piece()

        # ---- flat cross-j pipeline ----
        # One global group list: the S/PV/normalize pipeline flows straight
        # through j boundaries (no per-j drain, which used to leave the PE
        # thin for a few µs at each boundary and trip the HAM re-throttle).
        # Heads of a pair alternate (base partitions 0/64).  Extras (next
        # tile's projections + the previous j's out-projections) are woven
        # evenly through each j's groups.
        NJ = T // TQ
        sched = []   # (j, h, kcs, is_last_for_head, extras_after: list)
        for j in range(NJ):
            nk = 4 * (j + 1)
            groups = []
            for hp in range(2):
                for k0 in range(0, nk, 2):
                    for h in (2 * hp, 2 * hp + 1):
                        groups.append((j, h,
                                       [k for k in (k0, k0 + 1) if k < nk],
                                       k0 + 2 >= nk))
            extras = []
            if j + 1 < NJ:
                extras += proj_pieces(j + 1)
            if j >= 1:
                extras += outproj_pieces(j - 1)
            # weave extras shifted 2 groups early: the next tile's qT/kT
            # copies otherwise land exactly when the boundary S matmuls
            # need them, costing ~0.5us of PE wait per region boundary
            ei = 0
            for gi, g in enumerate(groups):
                want = min(len(extras),
                           (gi + 3) * len(extras) // len(groups))
                sched.append((g, extras[ei:want]))
                ei = want
        yts = {}         # (j, h) -> yt psum tile
        pending = None   # (j, h, st, info, is_last)
        norm_q = []      # (j, h) whose final PV is emitted, await normalize
        for (j, h, kcs, last), post in sched:
            st, info = s_group(j, h, kcs)
            if norm_q:
                nj, nh = norm_q.pop(0)
                normalize(nj, nh, yts.pop((nj, nh)))
            if pending is not None:
                pj, ph, pst, pinfo, plast = pending
                if (pj, ph) not in yts:
                    yts[(pj, ph)] = ps_yt.tile([65, TQ], f32, tag="yt",
                                               name=f"yt{pj}_{ph}")
                pv_group(pj, ph, pst, pinfo, yts[(pj, ph)], 4 * (pj + 1))
                if plast:
                    norm_q.append((pj, ph))
            pending = (j, h, st, info, last)
            for piece in post:
                piece()
        pj, ph, pst, pinfo, plast = pending
        if (pj, ph) not in yts:
            yts[(pj, ph)] = ps_yt.tile([65, TQ], f32, tag="yt",
                                       name=f"yt{pj}_{ph}")
        pv_group(pj, ph, pst, pinfo, yts[(pj, ph)], 4 * (pj + 1))
        drain_items = [(nj, nh, yts.pop((nj, nh))) for nj, nh in norm_q]
        drain_items.append((pj, ph, yts.pop((pj, ph))))
        normalize_drain(drain_items)

        for pi, piece in enumerate(outproj_pieces_epi(NJ - 1)):  # epilogue
            piece()
            # HAM filler: the epilogue's thin PE stream otherwise trips the
            # activity monitor back to half clock for its final ~8us.
            f_ps = ps_yt.tile([128, KC], f32, tag="yt", name=f"fill{pi}")
            for _ in range(10):
                nc.tensor.matmul(f_ps[:], tri_sb[:], tri_sb[:],
                                 start=True, stop=True)
    nc.compile()
    return nc


def make_in_maps(x, Wq, bq, Wk, bk, Wv, bv, Wp, bp):
    if MM_DT == "bfloat16":
        import ml_dtypes
        mmdt = ml_dtypes.bfloat16
    else:
        mmdt = np.float32
    x = np.asarray(x, np.float32)
    Wq, Wk, Wv, Wp = (np.asarray(w, np.float32) for w in (Wq, Wk, Wv, Wp))
    bq, bk, bv = (np.asarray(b, np.float32) for b in (bq, bk, bv))

    ones = np.ones((1, TQ), mmdt)
    kp = np.arange(KC)[:, None]
    qf = np.arange(KC)[None, :]
    tri = (qf >= kp).astype(mmdt)

    in_maps = []
    for core in range(N_CORES):
        b = core // 4
        hg = core % 4
        rows = slice(hg * CLOC, (hg + 1) * CLOC)
        wv_aug = np.zeros((C, VW), np.float32)
        bv_aug = np.zeros((1, VW), np.float32)
        for h in range(HLOC):
            wsl = slice(hg * CLOC + h * D, hg * CLOC + (h + 1) * D)
            wv_aug[:, h * 65:h * 65 + D] = Wv[wsl, :].T
            bv_aug[0, h * 65:h * 65 + D] = bv[wsl]
            bv_aug[0, h * 65 + D] = 1.0
        def pack(a):   # [8k, n] -> [128, k*n] partition-major chunks
            ch = a.reshape(-1, 128, a.shape[1])
            return np.concatenate(list(ch), axis=1)

        xT = x[b].T                         # [C, T]
        xc = xT.reshape(8, 128, T)
        in_maps.append({
            "xT0": np.concatenate([xc[c][:, :TQ] for c in range(8)],
                                  axis=1).astype(mmdt),
            "xTr": np.concatenate([xc[c][:, tb * TQ:(tb + 1) * TQ]
                                   for tb in range(1, T // TQ)
                                   for c in range(8)],
                                  axis=1).astype(mmdt),
            "wqP": pack(np.ascontiguousarray(Wq[rows, :].T)).astype(mmdt),
            "wkP": pack(np.ascontiguousarray(Wk[rows, :].T)).astype(mmdt),
            "wvP": pack(wv_aug).astype(mmdt),
            "wpP": pack(np.ascontiguousarray(Wp[:, rows].T)).astype(mmdt),
            "bq": np.ascontiguousarray(bq[rows][None, :]).astype(mmdt),
            "bk": np.ascontiguousarray(bk[rows][None, :]).astype(mmdt),
            "bv": bv_aug.astype(mmdt),
            "ones": ones,
            "tri": tri,
        })
    return in_maps


def kernel(x, Wq, bq, Wk, bk, Wv, bv, Wp, bp):
    from concourse.bass_utils import run_bass_kernel_spmd

    with_qk_bias = bool(np.any(np.asarray(bq)) or np.any(np.asarray(bk)))
    with_v_bias = bool(np.any(np.asarray(bv)))
    key = ("nc", with_qk_bias, with_v_bias)
    if key not in _CACHE:
        _CACHE[key] = build_nc(with_qk_bias, with_v_bias)
    nc = _CACHE[key]
    in_maps = make_in_maps(x, Wq, bq, Wk, bk, Wv, bv, Wp, bp)
    res = run_bass_kernel_spmd(nc, in_maps, core_ids=list(range(N_CORES)))
    out = np.zeros((B, T, C), np.float32)
    for core in range(N_CORES):
        out[core // 4] += np.asarray(res.results[core]["po"],
                                     dtype=np.float32)
    out += np.asarray(bp, np.float32)[None, None, :]
    return out



# revision 60
# speedup vs baseline: 1.0746x; 1.0746x over previous
"""Causal self-attention (B=2, T=2048, C=1024, H=16, D=64) on 8 trn2 cores.

Sharding: core = b*4 + hg  (data-parallel over batch b, tensor-parallel over
4 head-groups of 4 heads).  Each core computes q/k/v projections for its
256 head-dims, flash-style causal attention for its 4 heads, and a partial
output projection (its 256-column slice of Wp).  Partials are summed on the
host (the all-reduce), bias bp added there too.

Device layout notes:
  - All matmuls run in bfloat16 (measured 1.0 PE cycles/row on HW vs
    ~1.5 for float32r); partial outputs stored bf16 and summed f32 on
    the host.  Worst-case rel err ~4e-3 vs the 2e-2 gate.
  - Inputs are host-packed partition-major so every load is ONE
    contiguous dma_start (4-8KB descriptor rows).  Each dma_start costs
    ~600ns of serial HWDGE issue, so issue COUNT - not bytes - set the
    old startup latency.  All bulk loads ride the Sync ring in strict
    priority order (tri, x-tile0, wq, ...); ring FIFO gives earlier
    transfers full DMA bandwidth.
  - The PE HAM clock gate starts at 1.2GHz and doubles only after
    ~3.4us of sustained activity; dummy warm-up matmuls on tri burn the
    DMA head so the real stream starts at full clock, and filler
    matmuls in the thin epilogue keep it there.
  - qT/kT stored as [d', t] with 2 heads packed per 128 partitions.
  - v stored naturally [t, d'] with a ones-column appended per head
    (65 cols/head) so the PV matmul's output row 64 is the softmax
    denominator l[q] for free; the ones are GPSIMD memsets.
  - S^T tiles [k=128, q=512] land in PSUM, exp on ACT (scale=1/8 fused,
    no max-subtraction: |S|<~8 so exp is safe).  Diagonal chunks are
    column-restricted to the causal range; only the 128-wide triangle
    needs a mask multiply (DVE - GPSIMD tensor ops sem-stall the
    pipeline, ~600ns per event).
  - One flat software pipeline runs across all q-tiles (no per-j drain):
    S matmuls for group g+1 are emitted before exp/PV of group g, heads
    of a pair alternate, and projections for tile t+1 plus the previous
    tile's out-projections are woven in as PE filler.
  - Normalization: 1/l on DVE row 0, partition-broadcast on GPSIMD,
    DVE mul into yT - the PE stays out of it entirely.
"""
import numpy as np
from contextlib import ExitStack

B, T, C, H, D = 2, 2048, 1024, 16, 64
HLOC = 4            # heads per core
CLOC = HLOC * D     # 256 head-dims per core
VW = HLOC * 65      # v width with ones-columns: 260
N_CORES = 8
TQ = 512            # q tile width
KC = 128            # k chunk
NCC = C // 128      # 8 contraction chunks

MM_DT = "bfloat16"  # fp32r measures 1.5 cyc/row on HW; bf16 is 1.0

_CACHE = {}


def build_nc(with_qk_bias=True, with_v_bias=False):
    import concourse.tile as tile
    from concourse import bacc, mybir

    f32 = mybir.dt.float32
    fmm = getattr(mybir.dt, MM_DT)
    Exp = mybir.ActivationFunctionType.Exp

    nc = bacc.Bacc("TRN2", target_bir_lowering=False, debug=False,
                   num_devices=N_CORES)
    # Host-packed partition-major layouts: each input is a [128, *] tensor
    # whose rows match the SBUF destination exactly, so every load is ONE
    # dma_start with contiguous 1-4KB descriptor runs.  (Each dma_start
    # costs ~600ns of serial HWDGE issue time regardless of size, so issue
    # count — not bytes — governed the old kernel's startup latency.)
    xT0 = nc.dram_tensor("xT0", [128, NCC * TQ], fmm, kind="ExternalInput").ap()
    xTr = nc.dram_tensor("xTr", [128, NCC * (T - TQ)], fmm,
                         kind="ExternalInput").ap()
    wqP = nc.dram_tensor("wqP", [128, NCC * CLOC], fmm, kind="ExternalInput").ap()
    wkP = nc.dram_tensor("wkP", [128, NCC * CLOC], fmm, kind="ExternalInput").ap()
    wvP = nc.dram_tensor("wvP", [128, NCC * VW], fmm, kind="ExternalInput").ap()
    wpP = nc.dram_tensor("wpP", [128, 2 * C], fmm, kind="ExternalInput").ap()
    bq = nc.dram_tensor("bq", [1, CLOC], fmm, kind="ExternalInput").ap()
    bk = nc.dram_tensor("bk", [1, CLOC], fmm, kind="ExternalInput").ap()
    bv = nc.dram_tensor("bv", [1, VW], fmm, kind="ExternalInput").ap()
    ones = nc.dram_tensor("ones", [1, TQ], fmm, kind="ExternalInput").ap()
    tri = nc.dram_tensor("tri", [KC, KC], fmm, kind="ExternalInput").ap()
    po = nc.dram_tensor("po", [T, C], fmm, kind="ExternalOutput").ap()

    with tile.TileContext(nc) as tc, ExitStack() as ctx:
        persist = ctx.enter_context(tc.tile_pool(name="persist", bufs=1))
        pt_pool = ctx.enter_context(tc.tile_pool(name="pt", bufs=4))
        stage = ctx.enter_context(tc.tile_pool(name="stage", bufs=2))
        norm_pool = ctx.enter_context(tc.tile_pool(name="norm", bufs=4))
        ps_small = ctx.enter_context(
            tc.tile_pool(name="ps_small", bufs=2, space="PSUM"))
        ps_st = ctx.enter_context(
            tc.tile_pool(name="ps_st", bufs=2, space="PSUM"))
        ps_yt = ctx.enter_context(
            tc.tile_pool(name="ps_yt", bufs=2, space="PSUM"))

        # ---- persistent SBUF tensors + loads ----
        # xT_all columns: t-block-major [tb][c][t%TQ] — the whole tensor is
        # a contiguous copy of the (likewise packed) DRAM xT0/xTr, giving
        # full-row DMA descriptors; matmul rhs slices stay contiguous.
        xT_all = persist.tile([128, NCC * T], fmm, tag="xT", name="xT_all")
        wq_all = persist.tile([128, NCC * CLOC], fmm, tag="wq", name="wq_all")
        wk_all = persist.tile([128, NCC * CLOC], fmm, tag="wk", name="wk_all")
        wv_all = persist.tile([128, NCC * VW], fmm, tag="wv", name="wv_all")
        wp_all = persist.tile([128, 2 * C], fmm, tag="wp", name="wp_all")
        bq_sb = persist.tile([1, CLOC], fmm, tag="bq")
        bk_sb = persist.tile([1, CLOC], fmm, tag="bk")
        bv_sb = persist.tile([1, VW], fmm, tag="bv")
        ones_sb = persist.tile([1, TQ], fmm, tag="ones")
        tri_sb = persist.tile([KC, KC], fmm, tag="tri")
        qT_sb = [persist.tile([128, T], fmm, tag=f"qT{m}", name=f"qT{m}") for m in range(2)]
        kT_sb = [persist.tile([128, T], fmm, tag=f"kT{m}", name=f"kT{m}") for m in range(2)]
        v_sb = [persist.tile([128, VW], fmm, tag=f"v{t}", name=f"v{t}") for t in range(T // 128)]
        yT_sb = [persist.tile([128, T], fmm, tag=f"yT{m}", name=f"yT{m}") for m in range(2)]

        def xsl(c, pos, width):
            """xT_all slice for chunk c, x columns [pos, pos+width) —
            must stay within one TQ tile."""
            tb, off = divmod(pos, TQ)
            base = tb * NCC * TQ + c * TQ + off
            return xT_all[:, base:base + width]

        # All bulk loads go on ONE ring (Sync) in strict priority order —
        # ring FIFO means earlier transfers get full DMA bandwidth instead
        # of competing with later ones.  Only the tiny constants ride the
        # Scalar ring concurrently.
        HW = NCC * TQ // 2
        QH = NCC * CLOC // 2
        nc.sync.dma_start(tri_sb[:], tri[:])   # tiny; feeds PE warm-up MMs
        nc.sync.dma_start(xT_all[:, 0:HW], xT0[:, 0:HW])
        nc.sync.dma_start(wq_all[:, 0:QH], wqP[:, 0:QH])
        nc.sync.dma_start(wq_all[:, QH:2 * QH], wqP[:, QH:2 * QH])
        nc.sync.dma_start(xT_all[:, HW:2 * HW], xT0[:, HW:2 * HW])
        nc.sync.dma_start(wk_all[:], wkP[:, :])
        nc.sync.dma_start(wv_all[:], wvP[:, :])
        W = NCC * TQ
        for tb in range(1, T // TQ):
            nc.sync.dma_start(xT_all[:, tb * W:(tb + 1) * W],
                              xTr[:, (tb - 1) * W:tb * W])
        nc.sync.dma_start(wp_all[:], wpP[:, :])
        if with_v_bias:
            nc.scalar.dma_start(bv_sb[:], bv[:])
        if with_qk_bias or with_v_bias:
            nc.scalar.dma_start(ones_sb[:], ones[:])
        if with_qk_bias:
            nc.scalar.dma_start(bq_sb[:], bq[:])
            nc.scalar.dma_start(bk_sb[:], bk[:])

        # ---- interleaved emission: projections / attention / out-proj ----
        # The PE executes its queue in order, so emission order controls PE
        # density.  Attention for q-tile j only needs projections up to
        # t=j, so projections for t=j+1 and the out-projection for j-1 are
        # woven between attention groups of j to fill PE idle slots (keeps
        # the HAM clock-gate warm).
        def proj_qk(w_all, b_sb, dst, m, t):
            tsl = slice(t * TQ, (t + 1) * TQ)
            ps = ps_small.tile([128, TQ], f32, tag="ps_small")
            for c in range(NCC):
                nc.tensor.matmul(ps[:],
                                 w_all[:, c * CLOC + m * 128:
                                       c * CLOC + (m + 1) * 128],
                                 xsl(c, t * TQ, TQ),
                                 start=(c == 0),
                                 stop=(c == NCC - 1 and not with_qk_bias))
            if with_qk_bias:
                nc.tensor.matmul(ps[:], b_sb[0:1, m * 128:(m + 1) * 128],
                                 ones_sb[0:1, :],
                                 start=False, stop=True)
            nc.vector.tensor_copy(dst[m][:, tsl], ps[:])

        def proj_v(tt):
            ps = ps_small.tile([128, VW], f32, tag="ps_small")
            for c in range(NCC):
                nc.tensor.matmul(ps[:],
                                 xsl(c, tt * 128, 128),
                                 wv_all[:, c * VW:(c + 1) * VW],
                                 start=(c == 0),
                                 stop=(c == NCC - 1 and not with_v_bias))
            if with_v_bias:
                # supplies the ones-columns + v bias via the PE
                nc.tensor.matmul(ps[:], ones_sb[0:1, 0:128], bv_sb[:],
                                 start=False, stop=True)
            nc.vector.tensor_copy(v_sb[tt][:], ps[:])
            if not with_v_bias:
                # ones-columns written by the (otherwise idle) GPSIMD
                for h in range(HLOC):
                    nc.gpsimd.memset(v_sb[tt][:, h * 65 + 64:h * 65 + 65], 1.0)

        def proj_pieces(t):
            out = []
            for w_all, b_sb, dst in ((wq_all, bq_sb, qT_sb), (wk_all, bk_sb, kT_sb)):
                for m in range(2):
                    out.append(lambda w=w_all, b=b_sb, d=dst, mm=m:
                               proj_qk(w, b, d, mm, t))
            for tt in range(t * 4, t * 4 + 4):
                out.append(lambda x=tt: proj_v(x))
            return out

        def outproj_piece(tt, epilogue=False):
            ttsl = slice(tt * 128, (tt + 1) * 128)
            so = stage.tile([128, C], fmm, tag="so")
            for do in range(2):
                dsl = slice(do * TQ, (do + 1) * TQ)
                # epilogue: allocate from the attention-idle ps_st pool so
                # PE->DVE copy WAR chains don't throttle the final pieces
                pool = ps_st if epilogue else ps_small
                ops = pool.tile([128, TQ], f32,
                                tag="st" if epilogue else "ps_small")
                for m2 in range(2):
                    nc.tensor.matmul(ops[:], yT_sb[m2][:, ttsl],
                                     wp_all[:, m2 * C + do * TQ:
                                            m2 * C + (do + 1) * TQ],
                                     start=(m2 == 0), stop=(m2 == 1))
                nc.vector.tensor_copy(so[:, dsl], ops[:])
            # mid-run stores stay on Sync: a Scalar-issued store would
            # serialize behind queued EXP work and stall the PV pipeline.
            eng = nc.scalar if (epilogue and tt % 2) else nc.sync
            eng.dma_start(po[ttsl, :], so[:])

        def outproj_pieces_epi(j):
            return [lambda t=tt: outproj_piece(t, epilogue=True)
                    for tt in range(4 * j, 4 * j + 4)]

        def outproj_pieces(j):
            return [lambda t=tt: outproj_piece(t)
                    for tt in range(4 * j, 4 * j + 4)]

        def s_group(j, h, kcs):
            """Emit S matmuls for a k-chunk pair; return (st_tile, info)."""
            m, pr = h // 2, (h % 2) * 64
            st = ps_st.tile([128, 1024], f32, tag="st")
            info = []
            for i, kc in enumerate(kcs):
                coff = max(0, kc * KC - j * TQ)   # causal column offset
                nc.tensor.matmul(
                    st[:, i * TQ + coff:(i + 1) * TQ],
                    kT_sb[m][pr:pr + 64, kc * KC:(kc + 1) * KC],
                    qT_sb[m][pr:pr + 64, j * TQ + coff:(j + 1) * TQ],
                    start=True, stop=True)
                info.append((i, kc, coff))
            return st, info

        def pv_group(j, h, st, info, yt, nk):
            """exp + triangle mask + PV matmuls for a prepared S group."""
            pt = pt_pool.tile([128, 1024], fmm, tag="pt")
            runs = []
            for i, kc, coff in info:
                lo, hi = i * TQ + coff, (i + 1) * TQ
                if runs and runs[-1][1] == lo:
                    runs[-1][1] = hi
                else:
                    runs.append([lo, hi])
            for lo, hi in runs:
                nc.scalar.activation(pt[:, lo:hi], st[:, lo:hi], Exp, scale=0.125)
            for i, kc, coff in info:
                if kc >= 4 * j:   # diagonal chunk: mask the 128-wide triangle
                    lo = i * TQ + coff
                    nc.vector.tensor_mul(pt[:, lo:lo + KC], pt[:, lo:lo + KC],
                                         tri_sb[:])
            for i, kc, coff in info:
                lo = i * TQ + coff
                nc.tensor.matmul(
                    yt[0:65, coff:TQ] if coff else yt[:],
                    v_sb[kc][:, h * 65:(h + 1) * 65],
                    pt[:, lo:(i + 1) * TQ],
                    start=(kc == 0), stop=(kc == nk - 1))

        def normalize(j, h, yt):
            """yT[h slice, j] = yt[0:64] * broadcast(1/l).

            1/l on DVE (row 0 only), partition-broadcast on the idle
            GPSIMD — keeps the PE out of the normalization entirely.
            """
            m, pr = h // 2, (h % 2) * 64
            l_sb = norm_pool.tile([1, TQ], f32, tag="l")
            nc.vector.tensor_copy(l_sb[:], yt[64:65, :])
            r_sb = norm_pool.tile([1, TQ], f32, tag="r")
            nc.vector.reciprocal_approx_fast(r_sb[:], l_sb[:])
            bc_sb = stage.tile([64, TQ], f32, tag="bc")
            nc.gpsimd.partition_broadcast(bc_sb[:], r_sb[:])
            nc.vector.tensor_mul(yT_sb[m][pr:pr + 64, j * TQ:(j + 1) * TQ],
                                 yt[0:64, :], bc_sb[:])

        def normalize_drain(items):
            """Final normalizes: every epilogue m1 matmul waits on these, so
            latency matters.  Interleave the heads' chains so the GPSIMD
            broadcasts hide behind the other head's DVE ops (the DVE queue
            is strict FIFO — a serial emission stalls it on each
            broadcast)."""
            ls, rs, bcs = [], [], []
            for (j, h, yt) in items:
                l_sb = norm_pool.tile([1, TQ], f32, tag="l")
                nc.vector.tensor_copy(l_sb[:], yt[64:65, :])
                ls.append(l_sb)
            for l_sb in ls:
                r_sb = norm_pool.tile([1, TQ], f32, tag="r")
                nc.vector.reciprocal_approx_fast(r_sb[:], l_sb[:])
                rs.append(r_sb)
            for r_sb in rs:
                bc_sb = stage.tile([64, TQ], f32, tag="bc")
                nc.gpsimd.partition_broadcast(bc_sb[:], r_sb[:])
                bcs.append(bc_sb)
            for (j, h, yt), bc_sb in zip(items, bcs):
                m, pr = h // 2, (h % 2) * 64
                nc.vector.tensor_mul(
                    yT_sb[m][pr:pr + 64, j * TQ:(j + 1) * TQ],
                    yt[0:64, :], bc_sb[:])

        # ---- PE warm-up ----
        # The HAM clock gate needs ~3.4us of sustained PE activity before it
        # un-throttles (1.2 -> 2.4 GHz).  The PE sits idle waiting for input
        # DMA until ~13us; burn that window with dummy matmuls on tri_sb
        # (first tensor to land) so the real stream starts warm.
        warm_ps = ps_small.tile([128, KC], f32, tag="ps_small", name="warm")
        for _ in range(36):
            nc.tensor.matmul(warm_ps[:], tri_sb[:], tri_sb[:],
                             start=True, stop=True)

        for piece in proj_pieces(0):    # prologue
            piece()

        # ---- flat cross-j pipeline ----
        # One global group list: the S/PV/normalize pipeline flows straight
        # through j boundaries (no per-j drain, which used to leave the PE
        # thin for a few µs at each boundary and trip the HAM re-throttle).
        # Heads of a pair alternate (base partitions 0/64).  Extras (next
        # tile's projections + the previous j's out-projections) are woven
        # evenly through each j's groups.
        NJ = T // TQ
        sched = []   # (j, h, kcs, is_last_for_head, extras_after: list)
        for j in range(NJ):
            nk = 4 * (j + 1)
            groups = []
            for hp in range(2):
                for k0 in range(0, nk, 2):
                    for h in (2 * hp, 2 * hp + 1):
                        groups.append((j, h,
                                       [k for k in (k0, k0 + 1) if k < nk],
                                       k0 + 2 >= nk))
            extras = []
            if j + 1 < NJ:
                extras += proj_pieces(j + 1)
            if j >= 1:
                extras += outproj_pieces(j - 1)
            # even weave across all groups: region-tail extras density is
            # load-bearing — front-shifting (or reserving) extras empties
            # the tail and trips the HAM re-throttle, costing far more
            # than the ~0.5us boundary copy-wait it would save
            ei = 0
            for gi, g in enumerate(groups):
                want = (gi + 1) * len(extras) // len(groups)
                sched.append((g, extras[ei:want]))
                ei = want
        yts = {}         # (j, h) -> yt psum tile
        pending = None   # (j, h, st, info, is_last)
        norm_q = []      # (j, h) whose final PV is emitted, await normalize
        for (j, h, kcs, last), post in sched:
            st, info = s_group(j, h, kcs)
            if norm_q:
                nj, nh = norm_q.pop(0)
                normalize(nj, nh, yts.pop((nj, nh)))
            if pending is not None:
                pj, ph, pst, pinfo, plast = pending
                if (pj, ph) not in yts:
                    yts[(pj, ph)] = ps_yt.tile([65, TQ], f32, tag="yt",
                                               name=f"yt{pj}_{ph}")
                pv_group(pj, ph, pst, pinfo, yts[(pj, ph)], 4 * (pj + 1))
                if plast:
                    norm_q.append((pj, ph))
            pending = (j, h, st, info, last)
            for piece in post:
                piece()
        pj, ph, pst, pinfo, plast = pending
        if (pj, ph) not in yts:
            yts[(pj, ph)] = ps_yt.tile([65, TQ], f32, tag="yt",
                                       name=f"yt{pj}_{ph}")
        pv_group(pj, ph, pst, pinfo, yts[(pj, ph)], 4 * (pj + 1))
        drain_items = [(nj, nh, yts.pop((nj, nh))) for nj, nh in norm_q]
        drain_items.append((pj, ph, yts.pop((pj, ph))))
        normalize_drain(drain_items)

        for pi, piece in enumerate(outproj_pieces_epi(NJ - 1)):  # epilogue
            piece()
            # HAM filler: the epilogue's thin PE stream otherwise trips the
            # activity monitor back to half clock for its final ~8us.
            f_ps = ps_yt.tile([128, KC], f32, tag="yt", name=f"fill{pi}")
            for _ in range(10):
                nc.tensor.matmul(f_ps[:], tri_sb[:], tri_sb[:],
                                 start=True, stop=True)
    nc.compile()
    return nc


def make_in_maps(x, Wq, bq, Wk, bk, Wv, bv, Wp, bp):
    if MM_DT == "bfloat16":
        import ml_dtypes
        mmdt = ml_dtypes.bfloat16
    else:
        mmdt = np.float32
    x = np.asarray(x, np.float32)
    Wq, Wk, Wv, Wp = (np.asarray(w, np.float32) for w in (Wq, Wk, Wv, Wp))
    bq, bk, bv = (np.asarray(b, np.float32) for b in (bq, bk, bv))

    ones = np.ones((1, TQ), mmdt)
    kp = np.arange(KC)[:, None]
    qf = np.arange(KC)[None, :]
    tri = (qf >= kp).astype(mmdt)

    in_maps = []
    for core in range(N_CORES):
        b = core // 4
        hg = core % 4
        rows = slice(hg * CLOC, (hg + 1) * CLOC)
        wv_aug = np.zeros((C, VW), np.float32)
        bv_aug = np.zeros((1, VW), np.float32)
        for h in range(HLOC):
            wsl = slice(hg * CLOC + h * D, hg * CLOC + (h + 1) * D)
            wv_aug[:, h * 65:h * 65 + D] = Wv[wsl, :].T
            bv_aug[0, h * 65:h * 65 + D] = bv[wsl]
            bv_aug[0, h * 65 + D] = 1.0
        def pack(a):   # [8k, n] -> [128, k*n] partition-major chunks
            ch = a.reshape(-1, 128, a.shape[1])
            return np.concatenate(list(ch), axis=1)

        xT = x[b].T                         # [C, T]
        xc = xT.reshape(8, 128, T)
        in_maps.append({
            "xT0": np.concatenate([xc[c][:, :TQ] for c in range(8)],
                                  axis=1).astype(mmdt),
            "xTr": np.concatenate([xc[c][:, tb * TQ:(tb + 1) * TQ]
                                   for tb in range(1, T // TQ)
                                   for c in range(8)],
                                  axis=1).astype(mmdt),
            "wqP": pack(np.ascontiguousarray(Wq[rows, :].T)).astype(mmdt),
            "wkP": pack(np.ascontiguousarray(Wk[rows, :].T)).astype(mmdt),
            "wvP": pack(wv_aug).astype(mmdt),
            "wpP": pack(np.ascontiguousarray(Wp[:, rows].T)).astype(mmdt),
            "bq": np.ascontiguousarray(bq[rows][None, :]).astype(mmdt),
            "bk": np.ascontiguousarray(bk[rows][None, :]).astype(mmdt),
            "bv": bv_aug.astype(mmdt),
            "ones": ones,
            "tri": tri,
        })
    return in_maps


def kernel(x, Wq, bq, Wk, bk, Wv, bv, Wp, bp):
    from concourse.bass_utils import run_bass_kernel_spmd

    with_qk_bias = bool(np.any(np.asarray(bq)) or np.any(np.asarray(bk)))
    with_v_bias = bool(np.any(np.asarray(bv)))
    key = ("nc", with_qk_bias, with_v_bias)
    if key not in _CACHE:
        _CACHE[key] = build_nc(with_qk_bias, with_v_bias)
    nc = _CACHE[key]
    in_maps = make_in_maps(x, Wq, bq, Wk, bk, Wv, bv, Wp, bp)
    res = run_bass_kernel_spmd(nc, in_maps, core_ids=list(range(N_CORES)))
    out = np.zeros((B, T, C), np.float32)
    for core in range(N_CORES):
        out[core // 4] += np.asarray(res.results[core]["po"],
                                     dtype=np.float32)
    out += np.asarray(bp, np.float32)[None, None, :]
    return out

